# revision 1
# baseline (speedup 1.0000x reference)
"""Col2Octree scatter-add kernel for 8 Trainium2 NeuronCores.

Per core (column-sharded H): stream data tiles, PE-transpose each [64,128]
channel block to row format [128 updates, 64 ch]; for each 128-update tile,
read-modify-write a node-major accumulator table in DRAM via indirect DMA:
gather current rows at the tile's indices, combine duplicate indices inside
the tile with an equality-matrix matmul (so colliding writes all carry the
same value), add, scatter back. Tiles round-robin over NTAB independent
accumulator tables to shorten the serial RMW dependency chains.
Host: shards/pads inputs, sums the 8*NTAB node-major partials, transposes.
"""
import sys
import numpy as np

try:
    import concourse.bass as bass
except ImportError:
    sys.path.insert(0, "/opt/trn_rl_repo")
    import concourse.bass as bass

import concourse.bacc as bacc
import concourse.mybir as mybir
import concourse.tile as tile
from concourse import bass_utils
from concourse.masks import make_identity

F32 = mybir.dt.float32
F16 = mybir.dt.float16
I32 = mybir.dt.int32
OP = mybir.AluOpType


def cdiv(a, b):
    return -(-a // b)


class Cfg:
    def __init__(self, N=200000, ncores=8, Hc=25000, Hcp=25088, NTAB=4):
        self.C, self.K = 64, 27
        self.N, self.ncores, self.Hc, self.Hcp = N, ncores, Hc, Hcp
        assert Hcp % 128 == 0
        self.NBATCH = Hcp // 128
        self.NTAB = NTAB
        self.TROWS = cdiv(N + 1, 128) * 128      # table rows (dump at N)


def build_program(cfg):
    C, K = cfg.C, cfg.K
    nc = bacc.Bacc("TRN2")
    data = nc.dram_tensor("data", [C, K, cfg.Hcp], F32, kind="ExternalInput")
    oct_ = nc.dram_tensor("oct", [cfg.Hcp, K], I32, kind="ExternalInput")
    tabs = [nc.dram_tensor(f"tab{m}", [cfg.TROWS, C], F32,
                           kind="ExternalOutput") for m in range(cfg.NTAB)]

    with tile.TileContext(nc) as tc:
        with tc.tile_pool(name="const", bufs=1) as cpool, \
             tc.tile_pool(name="p1", bufs=3) as p1, \
             tc.tile_pool(name="rmw", bufs=8) as rmw, \
             tc.tile_pool(name="tp_ps", bufs=2, space="PSUM") as tpps, \
             tc.tile_pool(name="mm_ps", bufs=2, space="PSUM") as mmps:

            ident = cpool.tile([128, 128], F32, tag="ident")
            make_identity(nc, ident[:])

            # zero-init tables
            zcols = cfg.TROWS * C // 128 // 16
            zt = cpool.tile([128, zcols], F32, tag="zt")
            nc.vector.memset(zt[:], 0.0)
            for t in tabs:
                flat = t[:].rearrange("r c -> (r c)")
                for s in range(16):
                    B = 128 * zcols
                    nc.sync.dma_start(
                        out=flat[s * B:(s + 1) * B].rearrange(
                            "(p z) -> p z", p=128),
                        in_=zt[:])

            for hb in range(cfg.NBATCH):
                dt_ = p1.tile([64, K * 128], F32, tag="dt")
                nc.sync.dma_start(
                    out=dt_[:].rearrange("c (k h) -> c k h", k=K),
                    in_=data[:, :, hb * 128:(hb + 1) * 128])
                ot = p1.tile([128, K], I32, tag="ot")
                nc.sync.dma_start(out=ot[:], in_=oct_[hb * 128:(hb + 1) * 128, :])
                neg = p1.tile([128, K], I32, tag="neg")
                nc.vector.tensor_scalar(out=neg[:], in0=ot[:], scalar1=0,
                                        scalar2=None, op0=OP.is_lt)
                idxp = p1.tile([128, K], I32, tag="idxp")
                nc.vector.scalar_tensor_tensor(
                    out=idxp[:], in0=neg[:], scalar=cfg.N + 1, in1=ot[:],
                    op0=OP.mult, op1=OP.add)
                idxf = p1.tile([128, K], F32, tag="idxf")
                nc.vector.tensor_copy(out=idxf[:], in_=idxp[:])

                # transpose the 27 [64,128] channel blocks -> rows fp16
                stg = p1.tile([128, K * 64], F16, tag="stg")
                for q in range(cdiv(K, 7)):
                    ks = list(range(7 * q, min(7 * q + 7, K)))
                    tp = tpps.tile([128, len(ks) * 64], F32, tag="tp")
                    for i, k in enumerate(ks):
                        nc.tensor.transpose(
                            out=tp[:, i * 64:(i + 1) * 64],
                            in_=dt_[:, k * 128:(k + 1) * 128],
                            identity=ident[:64, :64])
                    nc.vector.tensor_copy(
                        out=stg[:, ks[0] * 64:(ks[-1] + 1) * 64], in_=tp[:])

                for k in range(K):
                    m = (hb * K + k) % cfg.NTAB
                    tabm = tabs[m]
                    # idx column transposed (for the equality matrix)
                    ixT = mmps.tile([128, 128], F32, tag="ixT")
                    nc.tensor.transpose(
                        out=ixT[:],
                        in_=idxf[:, k:k + 1].to_broadcast([128, 128]),
                        identity=ident[:])
                    ixTs = rmw.tile([128, 128], F32, tag="ixTs")
                    nc.vector.tensor_copy(out=ixTs[:], in_=ixT[:])
                    S = rmw.tile([128, 128], F16, tag="S")
                    nc.vector.tensor_tensor(
                        out=S[:],
                        in0=idxf[:, k:k + 1].to_broadcast([128, 128]),
                        in1=ixTs[:], op=OP.is_equal)
                    cur = rmw.tile([128, C], F32, tag="cur")
                    nc.gpsimd.indirect_dma_start(
                        out=cur[:], out_offset=None, in_=tabm[:],
                        in_offset=bass.IndirectOffsetOnAxis(
                            ap=idxp[:, k:k + 1], axis=0))
                    comb = mmps.tile([128, C], F32, tag="comb")
                    nc.tensor.matmul(comb[:], lhsT=S[:],
                                     rhs=stg[:, k * 64:(k + 1) * 64],
                                     start=True, stop=True)
                    new = rmw.tile([128, C], F32, tag="new")
                    nc.vector.tensor_tensor(out=new[:], in0=cur[:],
                                            in1=comb[:], op=OP.add)
                    nc.gpsimd.indirect_dma_start(
                        out=tabm[:],
                        out_offset=bass.IndirectOffsetOnAxis(
                            ap=idxp[:, k:k + 1], axis=0),
                        in_=new[:], in_offset=None)

    nc.compile()
    return nc


_CACHED = {}


def _get_program(cfg):
    key = (cfg.N, cfg.ncores, cfg.Hcp, cfg.NTAB)
    if key not in _CACHED:
        _CACHED[key] = build_program(cfg)
    return _CACHED[key]


def shard_inputs(cfg, data_in, octree):
    maps = []
    for c in range(cfg.ncores):
        dc = np.zeros((cfg.C, cfg.K, cfg.Hcp), np.float32)
        dc[:, :, : cfg.Hc] = data_in[:, :, c * cfg.Hc:(c + 1) * cfg.Hc]
        oc = np.empty((cfg.Hcp, cfg.K), np.int32)
        oc[: cfg.Hc] = octree[c * cfg.Hc:(c + 1) * cfg.Hc]
        if cfg.Hcp > cfg.Hc:
            hpad = np.arange(cfg.Hc, cfg.Hcp)
            oc[cfg.Hc:] = (((hpad[:, None] * cfg.K
                             + np.arange(cfg.K)[None, :]) * 37)
                           % cfg.N).astype(np.int32)
        maps.append({"data": dc, "oct": oc})
    return maps


def unshard_output(cfg, results):
    acc = None
    for r in results:
        for m in range(cfg.NTAB):
            t = r[f"tab{m}"]
            acc = t.astype(np.float32) if acc is None else acc + t
    return np.ascontiguousarray(acc[: cfg.N, :].T)


def kernel(data_in, octree):
    data_in = np.asarray(data_in, dtype=np.float32)
    octree = np.asarray(octree, dtype=np.int32)
    cfg = Cfg()
    nc = _get_program(cfg)
    in_maps = shard_inputs(cfg, data_in, octree)
    res = bass_utils.run_bass_kernel_spmd(
        nc, in_maps, core_ids=list(range(cfg.ncores)))
    return unshard_output(cfg, res.results)

